# revision 1
# baseline (speedup 1.0000x reference)
"""Causal self-attention on 8 NeuronCores (TRN2), tensor-parallel over heads.

Reference: y = proj(softmax(causal(Q K^T / sqrt(64))) V) with
B=4, T=2048, D=1024, H=16 heads, head_dim=64.

Sharding: each core owns 2 heads (a 128-column slice of the Q/K/V
projections and the matching 128 rows of w_proj) for all batches. Each
core emits a partial [B*T, D] output; the host sums the 8 partials
(row-parallel matmul unshard) and reshapes to [B, T, D].
"""

import sys

for _p in ("/opt/trn_rl_repo",):
    if _p not in sys.path:
        sys.path.insert(0, _p)

import numpy as np

import concourse.bass as bass
import concourse.bacc as bacc
import concourse.mybir as mybir
from concourse import tile
from concourse.bass_utils import run_bass_kernel_spmd
from concourse.masks import make_identity

B, T, D, H = 4, 2048, 1024, 16
HD = D // H           # 64 head dim
NCORES = 8
HPC = H // NCORES     # 2 heads per core
CW = HPC * HD         # 128: per-core qkv column slice width
BT = B * T            # 8192 tokens
KC = D // 128         # 8 contraction chunks for the qkv projection
NQ = 512              # query chunk
NG = NQ // 128        # 4 key-tiles per S^T group
F32 = mybir.dt.float32
F32R = mybir.dt.float32r
EXP = mybir.ActivationFunctionType.Exp

VST = HPC * (HD + 1)  # 130: V tile stride (per head: 64 cols + ones col)


def build_kernel():
    nc = bacc.Bacc("TRN2", target_bir_lowering=False, debug=False)

    xT = nc.dram_tensor("xT", [D, BT], F32R, kind="ExternalInput")
    # wqkv packed on host as [128, KC, 3*CW]: (kc,:) = rows kc*128..+128 of
    # [w_q_slice | w_k_slice | w_v_slice]
    wqkv = nc.dram_tensor("wqkv", [128, KC * 3 * CW], F32R, kind="ExternalInput")
    wp = nc.dram_tensor("wp", [CW, D], F32R, kind="ExternalInput")
    out = nc.dram_tensor("out", [BT, D], F32, kind="ExternalOutput")

    with tile.TileContext(nc) as tc:
        _body(tc, xT.ap(), wqkv.ap(), wp.ap(), out.ap())
    nc.compile()
    return nc


def _body(tc, xT, wqkv, wp, out):
    nc = tc.nc
    with (
        tc.tile_pool(name="const", bufs=1) as const,
        tc.tile_pool(name="xin", bufs=2) as xin,
        tc.tile_pool(name="qk", bufs=2) as qkpool,
        tc.tile_pool(name="vb", bufs=2) as vbpool,
        tc.tile_pool(name="vs", bufs=2) as vspool,
        tc.tile_pool(name="pt", bufs=3) as ptpool,
        tc.tile_pool(name="ptd", bufs=2) as ptdpool,
        tc.tile_pool(name="yt", bufs=2) as ytpool,
        tc.tile_pool(name="dn", bufs=2) as dnpool,
        tc.tile_pool(name="os", bufs=2) as ospool,
        tc.tile_pool(name="pst", bufs=2, space="PSUM") as pst,
        tc.tile_pool(name="pav", bufs=1, space="PSUM") as pav,
        tc.tile_pool(name="psm", bufs=2, space="PSUM") as psm,
    ):
        # ---- constants ----
        wq_sb = const.tile([128, KC, 3 * CW], F32R, tag="wqkv")
        nc.sync.dma_start(wq_sb[:], wqkv.rearrange("p (k c) -> p k c", k=KC))
        wp_sb = const.tile([CW, D], F32R, tag="wp")
        nc.sync.dma_start(wp_sb[:], wp[:])
        ident = const.tile([128, 128], F32, tag="ident")
        make_identity(nc, ident[:])
        ones32 = const.tile([128, (T // 128) * HPC], F32, tag="ones32")
        nc.gpsimd.memset(ones32[:], 1.0)
        scale = 1.0 / float(np.sqrt(HD))

        def qkv_proj(b, qt, kt, vb):
            tok0 = b * T
            for ch in range(T // NQ):
                xt = xin.tile([128, KC, NQ], F32R, tag="xt")
                for kc in range(KC):
                    nc.sync.dma_start(
                        xt[:, kc, :],
                        xT[kc * 128 : (kc + 1) * 128,
                           tok0 + ch * NQ : tok0 + (ch + 1) * NQ],
                    )
                # Q^T and K^T m-tiles
                for m, dst in ((0, qt), (1, kt)):
                    ps = psm.tile([128, NQ], F32, tag="ps")
                    for kc in range(KC):
                        nc.tensor.matmul(
                            ps[:],
                            wq_sb[:, kc, m * CW : (m + 1) * CW],
                            xt[:, kc, :],
                            start=(kc == 0),
                            stop=(kc == KC - 1),
                        )
                    nc.vector.tensor_copy(dst[:, ch * NQ : (ch + 1) * NQ], ps[:])
                # V^T m-tile, then transpose into token-major layout
                ps = psm.tile([128, NQ], F32, tag="ps")
                for kc in range(KC):
                    nc.tensor.matmul(
                        ps[:],
                        wq_sb[:, kc, 2 * CW : 3 * CW],
                        xt[:, kc, :],
                        start=(kc == 0),
                        stop=(kc == KC - 1),
                    )
                vs = vspool.tile([128, NQ], F32, tag="vs")
                nc.vector.tensor_copy(vs[:], ps[:])
                pt2 = psm.tile([128, NQ], F32, tag="ps")
                for q in range(NG):
                    nc.tensor.transpose(
                        pt2[:, q * 128 : (q + 1) * 128],
                        vs[:, q * 128 : (q + 1) * 128],
                        ident[:],
                    )
                # pt2 holds [tok 128][tile q: h0 64 | h1 64]; scatter into vb
                # (col 0 of each 65-col head block is the ones column)
                dstv = bass.AP(
                    vb.tensor,
                    vb[:].offset + ch * NG * VST,
                    [vb[:].ap[0], [VST, NG], [HD + 1, HPC], [1, HD]],
                )
                srcv = pt2[:].rearrange("p (t h d) -> p t h d", t=NG, h=HPC)
                nc.vector.tensor_copy(dstv, srcv)
            # ones columns (denominator trick): col 65*j + HD of vb
            onesv = bass.AP(
                vb.tensor,
                vb[:].offset + HD,
                [vb[:].ap[0], [HD + 1, (T // 128) * HPC]],
            )
            nc.vector.tensor_copy(onesv, ones32[:])

        def finalize_norm(yt, jq, ytu):
            # divide O^T rows by the denominator row (broadcast to 64 parts)
            q0 = jq * NQ
            dn = dnpool.tile([1, HPC * NQ], F32, tag="dn")
            nc.vector.reciprocal(dn[:], ytu[HD : HD + 1, :])
            dnb = dnpool.tile([HD, HPC * NQ], F32, tag="dnb")
            nc.gpsimd.partition_broadcast(dnb[:], dn[:])
            for h in range(HPC):
                nc.vector.tensor_mul(
                    yt[h * HD : (h + 1) * HD, q0 : q0 + NQ],
                    ytu[0:HD, h * NQ : (h + 1) * NQ],
                    dnb[:, h * NQ : (h + 1) * NQ],
                )

        def attention(b, qt, kt, vb, yt):
            # Both heads processed together per kk-tile: h0 lives in SBUF
            # partitions 0-63, h1 in 64-127, so the S^T matmul pairs land on
            # PE row-tiles (64,128)@(0,0) and @(64,0) and can overlap.
            pending = None
            for jq in range(T // NQ):
                q0 = jq * NQ
                av0 = pav.tile([128, NQ], F32, tag="av0")
                av1 = pav.tile([128, NQ], F32, tag="av1")
                avs = [av0, av1]
                nkk = NG * (jq + 1)
                diag0 = NG * jq
                for kk in range(nkk):
                    i = kk - diag0          # >= 0 on the diagonal run
                    c0 = max(i, 0) * 128    # first valid q col in this chunk
                    w = NQ - c0
                    st = pst.tile([128, HPC * NQ], F32, tag="st")
                    for h in range(HPC):
                        nc.tensor.matmul(
                            st[:, h * NQ + c0 : (h + 1) * NQ],
                            kt[h * HD : (h + 1) * HD, kk * 128 : (kk + 1) * 128],
                            qt[h * HD : (h + 1) * HD, q0 + c0 : q0 + NQ],
                            start=True,
                            stop=True,
                        )
                    ptk = ptpool.tile([128, HPC * NQ], F32R, tag="pt")
                    stv = bass.AP(st.tensor, st[:].offset + c0,
                                  [st[:].ap[0], [NQ, HPC], [1, w]])
                    ptv = bass.AP(ptk.tensor, ptk[:].offset + c0,
                                  [ptk[:].ap[0], [NQ, HPC], [1, w]])
                    nc.scalar.activation(ptv, stv, EXP, scale=scale)
                    if i >= 0:
                        # zero q < kpart inside the 128-wide diagonal block
                        tri = bass.AP(ptk.tensor, ptk[:].offset + c0,
                                      [ptk[:].ap[0], [NQ, HPC], [1, 128]])
                        nc.gpsimd.affine_select(
                            out=tri,
                            in_=tri,
                            pattern=[[0, HPC], [1, 128]],
                            channel_multiplier=-1,
                            base=0,
                            compare_op=mybir.AluOpType.is_ge,
                            fill=0.0,
                        )
                    for h in range(HPC):
                        nc.tensor.matmul(
                            avs[h][0 : HD + 1, c0:NQ],
                            vb[:, kk * VST + h * (HD + 1) :
                                 kk * VST + (h + 1) * (HD + 1)],
                            ptk[:, h * NQ + c0 : (h + 1) * NQ],
                            start=(kk == 0),
                            stop=(kk == nkk - 1),
                        )
                # evacuate PSUM promptly (frees the av slots); rows 0..63 are
                # the unnormalized O^T, row 64 the denominator
                ytu = dnpool.tile([HD + 1, HPC * NQ], F32, tag="ytu")
                for h in range(HPC):
                    nc.vector.tensor_copy(
                        ytu[:, h * NQ : (h + 1) * NQ], avs[h][0 : HD + 1, :]
                    )
                if pending is not None:
                    finalize_norm(yt, *pending)
                pending = (jq, ytu)
            finalize_norm(yt, *pending)

        def out_proj(b, yt):
            tok0 = b * T
            for tt in range(T // 128):
                os_ = ospool.tile([128, D], F32, tag="os")
                for nn in range(D // NQ):
                    pp = psm.tile([128, NQ], F32, tag="ps")
                    nc.tensor.matmul(
                        pp[:],
                        yt[:, tt * 128 : (tt + 1) * 128],
                        wp_sb[:, nn * NQ : (nn + 1) * NQ],
                        start=True,
                        stop=True,
                    )
                    nc.vector.tensor_copy(os_[:, nn * NQ : (nn + 1) * NQ], pp[:])
                nc.sync.dma_start(
                    out[tok0 + tt * 128 : tok0 + (tt + 1) * 128, :], os_[:]
                )

        # out_proj(b) is emitted after qkv_proj(b+1) so the PE always has
        # independent work while batch b's last normalization drains.
        prev = None
        for b in range(B):
            qt = qkpool.tile([128, T], F32R, tag="qt")
            kt = qkpool.tile([128, T], F32R, tag="kt")
            vb = vbpool.tile([128, (T // 128) * VST], F32R, tag="vb")
            yt = ytpool.tile([128, T], F32R, tag="yt")
            qkv_proj(b, qt, kt, vb)
            if prev is not None:
                out_proj(*prev)
            attention(b, qt, kt, vb, yt)
            prev = (b, yt)
        out_proj(*prev)


_NC_CACHE = None


def kernel(x: np.ndarray, w_attn: np.ndarray, w_proj: np.ndarray) -> np.ndarray:
    global _NC_CACHE
    if _NC_CACHE is None:
        _NC_CACHE = build_kernel()
    nc = _NC_CACHE

    x = np.asarray(x, dtype=np.float32)
    w_attn = np.asarray(w_attn, dtype=np.float32)
    w_proj = np.asarray(w_proj, dtype=np.float32)

    xT = np.ascontiguousarray(x.reshape(BT, D).T)  # [D, BT]

    in_maps = []
    for c in range(NCORES):
        c0 = c * CW
        wq = w_attn[:, c0 : c0 + CW]
        wk = w_attn[:, D + c0 : D + c0 + CW]
        wv = w_attn[:, 2 * D + c0 : 2 * D + c0 + CW]
        wslice = np.concatenate([wq, wk, wv], axis=1)          # [D, 3*CW]
        wpacked = np.ascontiguousarray(
            wslice.reshape(KC, 128, 3 * CW).transpose(1, 0, 2)
        ).reshape(128, KC * 3 * CW)
        wpc = np.ascontiguousarray(w_proj[c0 : c0 + CW, :])    # [CW, D]
        in_maps.append({"xT": xT, "wqkv": wpacked, "wp": wpc})

    res = run_bass_kernel_spmd(nc, in_maps, core_ids=list(range(NCORES)))
    acc = np.zeros((BT, D), dtype=np.float32)
    for r in res.results:
        acc += r["out"]
    return acc.reshape(B, T, D)


if __name__ == "__main__":
    inputs = {
        "x": np.random.randn(B, T, D).astype(np.float32),
        "w_attn": (np.random.randn(D, 3 * D) / np.sqrt(D)).astype(np.float32),
        "w_proj": (np.random.randn(D, D) / np.sqrt(D)).astype(np.float32),
    }
    y = kernel(**inputs)
    print(y.shape, y.dtype)



# revision 10
# speedup vs baseline: 1.4581x; 1.4581x over previous
"""Causal self-attention on 8 NeuronCores (TRN2), batch x head-group hybrid.

Reference: y = proj(softmax(causal(Q K^T / sqrt(64))) V) with
B=4, T=2048, D=1024, H=16 heads, head_dim=64.

Sharding: core c owns batch c//2 and head-group c%2 (8 heads = 4 head
pairs). Each core reads only its batch's x (pre-packed bf16 on host),
computes QKV for its 512 qkv columns, runs attention for its 4 head
pairs, and projects through its 512 rows of w_proj, emitting a
[T, D] fp32 partial. The host sums the 2 partials per batch.

All matmuls run in bf16 (1 cycle/row on the PE vs ~1.8 effective for
fp32r) with fp32 PSUM accumulation; softmax denominators use the
fast DVE reciprocal (tolerance is 2e-2).
"""

import sys

for _p in ("/opt/trn_rl_repo",):
    if _p not in sys.path:
        sys.path.insert(0, _p)

import ml_dtypes
import numpy as np

import concourse.bass as bass
import concourse.bacc as bacc
import concourse.mybir as mybir
from concourse import tile
from concourse.bass_utils import run_bass_kernel_spmd
from concourse.masks import make_identity

B, T, D, H = 4, 2048, 1024, 16
HD = D // H           # 64 head dim
NCORES = 8
GH = 8                # heads per core (head group)
NP = GH // 2          # 4 head pairs per core
CW = GH * HD          # 512: per-core qkv column slice width per matrix
KC = D // 128         # 8 contraction chunks for the qkv projection
NM = 3 * CW // 128    # 12 qkv projection m-tiles (4 Q, 4 K, 4 V pairs)
NCH = T // 512        # 4 token chunks
NQ = 512              # query chunk
NG = NQ // 128        # 4 key-tiles per S^T group
F32 = mybir.dt.float32
BF16 = mybir.dt.bfloat16
EXP = mybir.ActivationFunctionType.Exp

VST = 2 * (HD + 1)    # 130: per-pair V tile stride (per head: 64 cols + ones)


def build_kernel():
    nc = bacc.Bacc("TRN2", target_bir_lowering=False, debug=False)

    # host-packed layouts (see make_in_maps):
    #   xb [128, ch, kc, 512]  bf16 -- x[b]^T chunked for the qkv matmuls
    #   wq [128, m, kc, 128]   bf16 -- qkv weight m-tiles (m: 4 Q, 4 K, 4 V)
    #   wp [128, pair, 1024]   bf16 -- w_proj rows for this head group
    xb = nc.dram_tensor("xb", [128, NCH * KC * NQ], BF16, kind="ExternalInput")
    wq = nc.dram_tensor("wq", [128, NM * KC * 128], BF16, kind="ExternalInput")
    wp = nc.dram_tensor("wp", [128, NP * D], BF16, kind="ExternalInput")
    out = nc.dram_tensor("out", [T, D], F32, kind="ExternalOutput")

    with tile.TileContext(nc) as tc:
        _body(tc, xb.ap(), wq.ap(), wp.ap(), out.ap())
    nc.compile()
    return nc


def _body(tc, xb, wq, wp, out):
    nc = tc.nc
    with (
        tc.tile_pool(name="const", bufs=1) as const,
        tc.tile_pool(name="vs", bufs=2) as vspool,
        tc.tile_pool(name="pt", bufs=3) as ptpool,
        tc.tile_pool(name="ytu", bufs=2) as ytupool,
        tc.tile_pool(name="dn", bufs=2) as dnpool,
        tc.tile_pool(name="os", bufs=2) as ospool,
        tc.tile_pool(name="pst", bufs=2, space="PSUM") as pst,
        tc.tile_pool(name="pav", bufs=1, space="PSUM") as pav,
        tc.tile_pool(name="psm", bufs=2, space="PSUM") as psm,
    ):
        # ---- persistent tiles ----
        wq_sb = const.tile([128, NM, KC, 128], BF16, tag="wq")
        xb_sb = const.tile([128, NCH, KC, NQ], BF16, tag="xb")
        wp_sb = const.tile([128, NP, D], BF16, tag="wp")
        qt = const.tile([128, NP, T], BF16, tag="qt")
        kt = const.tile([128, NP, T], BF16, tag="kt")
        vb = const.tile([128, NP, T // 128, VST], BF16, tag="vb")
        yt = const.tile([128, NP, T], BF16, tag="yt")
        ident = const.tile([128, 128], BF16, tag="ident")
        ones = const.tile([128, 128], BF16, tag="ones")

        wqv = wq.rearrange("p (m k t) -> p m k t", m=NM, k=KC)
        xbv = xb.rearrange("p (c k t) -> p c k t", c=NCH, k=KC)
        # first m-tile + first token chunk first so compute starts early
        nc.sync.dma_start(wq_sb[:, 0], wqv[:, 0])
        nc.sync.dma_start(xb_sb[:, 0], xbv[:, 0])
        for m in range(1, NM):
            nc.sync.dma_start(wq_sb[:, m], wqv[:, m])
        nc.sync.dma_start(wp_sb[:], wp.rearrange("p (h t) -> p h t", h=NP))
        for c in range(1, NCH):
            nc.sync.dma_start(xb_sb[:, c], xbv[:, c])

        make_identity(nc, ident[:])
        nc.gpsimd.memset(ones[:], 1.0)
        # ones columns (denominator trick): col 65*j + 64 of every vb tile
        onesv = bass.AP(
            vb.tensor, vb[:].offset + HD, [vb[:].ap[0], [HD + 1, NP * 32]]
        )
        nc.vector.tensor_copy(onesv, ones[:])
        scale = 1.0 / float(np.sqrt(HD))

        def qkv_proj():
            for ch in range(NCH):
                t0 = ch * NQ
                for m in range(NM):
                    ps = psm.tile([128, NQ], F32, tag="ps")
                    for kc in range(KC):
                        nc.tensor.matmul(
                            ps[:],
                            wq_sb[:, m, kc, :],
                            xb_sb[:, ch, kc, :],
                            start=(kc == 0),
                            stop=(kc == KC - 1),
                        )
                    if m < NP:
                        nc.vector.tensor_copy(qt[:, m, t0 : t0 + NQ], ps[:])
                    elif m < 2 * NP:
                        nc.vector.tensor_copy(kt[:, m - NP, t0 : t0 + NQ], ps[:])
                    else:
                        pv = m - 2 * NP
                        vs = vspool.tile([128, NQ], BF16, tag="vs")
                        nc.vector.tensor_copy(vs[:], ps[:])
                        # borrow an st slot: the pst pool is idle during the
                        # qkv phase, and this keeps the two ps slots free so
                        # consecutive accumulation groups overlap
                        pt2 = pst.tile([128, NQ], BF16, tag="st")
                        for q in range(NG):
                            nc.tensor.transpose(
                                pt2[:, q * 128 : (q + 1) * 128],
                                vs[:, q * 128 : (q + 1) * 128],
                                ident[:],
                            )
                        # pt2: [tok 128][tile q: h0 64 | h1 64] -> vb slots
                        dstv = bass.AP(
                            vb.tensor,
                            vb[:].offset + (pv * (T // 128) + ch * NG) * VST,
                            [vb[:].ap[0], [VST, NG], [HD + 1, 2], [1, HD]],
                        )
                        srcv = pt2[:].rearrange("p (t h d) -> p t h d", t=NG, h=2)
                        nc.vector.tensor_copy(dstv, srcv)

        def finalize_norm(p, jq, ytu):
            # divide O^T rows by the denominator row (broadcast to 64 parts)
            q0 = jq * NQ
            # the custom-DVE reciprocal and partition_broadcast only operate
            # on physical partition 0, so first move the denominator row
            # (partition 64) down with a plain scalar-engine copy
            dn0 = dnpool.tile([1, 2 * NQ], F32, tag="dn0")
            nc.scalar.copy(dn0[:], ytu[HD : HD + 1, :])
            dnr0 = dnpool.tile([1, 2 * NQ], F32, tag="dnr0")
            nc.vector.reciprocal_approx_fast(dnr0[:], dn0[:])
            dnr = dnpool.tile([HD, 2 * NQ], F32, tag="dnr")
            nc.gpsimd.partition_broadcast(dnr[:], dnr0[:])
            for h in range(2):
                nc.vector.tensor_mul(
                    yt[h * HD : (h + 1) * HD, p, q0 : q0 + NQ],
                    ytu[0:HD, h * NQ : (h + 1) * NQ],
                    dnr[:, h * NQ : (h + 1) * NQ],
                )

        def attention(p):
            # Both heads of the pair processed together per kk-tile: h0 in
            # SBUF partitions 0-63, h1 in 64-127, so the S^T matmul pairs
            # land on PE row-tiles (64,128)@(0,0) and @(64,0) and overlap.
            pending = None
            for jq in range(T // NQ):
                q0 = jq * NQ
                av0 = pav.tile([128, NQ], F32, tag="av0")
                av1 = pav.tile([128, NQ], F32, tag="av1")
                avs = [av0, av1]
                nkk = NG * (jq + 1)
                diag0 = NG * jq
                for kk in range(nkk):
                    i = kk - diag0          # >= 0 on the diagonal run
                    c0 = max(i, 0) * 128    # first valid q col in this chunk
                    w = NQ - c0
                    st = pst.tile([128, 2 * NQ], F32, tag="st")
                    for h in range(2):
                        nc.tensor.matmul(
                            st[:, h * NQ + c0 : (h + 1) * NQ],
                            kt[h * HD : (h + 1) * HD, p,
                               kk * 128 : (kk + 1) * 128],
                            qt[h * HD : (h + 1) * HD, p, q0 + c0 : q0 + NQ],
                            start=True,
                            stop=True,
                        )
                    ptk = ptpool.tile([128, 2 * NQ], BF16, tag="pt")
                    stv = bass.AP(st.tensor, st[:].offset + c0,
                                  [st[:].ap[0], [NQ, 2], [1, w]])
                    ptv = bass.AP(ptk.tensor, ptk[:].offset + c0,
                                  [ptk[:].ap[0], [NQ, 2], [1, w]])
                    nc.scalar.activation(ptv, stv, EXP, scale=scale)
                    if i >= 0:
                        # zero q < kpart inside the 128-wide diagonal block
                        tri = bass.AP(ptk.tensor, ptk[:].offset + c0,
                                      [ptk[:].ap[0], [NQ, 2], [1, 128]])
                        nc.gpsimd.affine_select(
                            out=tri,
                            in_=tri,
                            pattern=[[0, 2], [1, 128]],
                            channel_multiplier=-1,
                            base=0,
                            compare_op=mybir.AluOpType.is_ge,
                            fill=0.0,
                        )
                    for h in range(2):
                        nc.tensor.matmul(
                            avs[h][0 : HD + 1, c0:NQ],
                            vb[:, p, kk, h * (HD + 1) : (h + 1) * (HD + 1)],
                            ptk[:, h * NQ + c0 : (h + 1) * NQ],
                            start=(kk == 0),
                            stop=(kk == nkk - 1),
                        )
                # evacuate PSUM promptly (frees the av slots); rows 0..63 are
                # the unnormalized O^T, row 64 the denominator
                ytu = ytupool.tile([HD + 1, 2 * NQ], F32, tag="ytu")
                for h in range(2):
                    nc.vector.tensor_copy(
                        ytu[:, h * NQ : (h + 1) * NQ], avs[h][0 : HD + 1, :]
                    )
                if pending is not None:
                    finalize_norm(p, *pending)
                pending = (jq, ytu)
            finalize_norm(p, *pending)

        def out_proj():
            for tt in range(T // 128):
                os_ = ospool.tile([128, D], F32, tag="os")
                for nn in range(D // NQ):
                    pp = psm.tile([128, NQ], F32, tag="ps")
                    for p in range(NP):
                        nc.tensor.matmul(
                            pp[:],
                            yt[:, p, tt * 128 : (tt + 1) * 128],
                            wp_sb[:, p, nn * NQ : (nn + 1) * NQ],
                            start=(p == 0),
                            stop=(p == NP - 1),
                        )
                    nc.vector.tensor_copy(os_[:, nn * NQ : (nn + 1) * NQ], pp[:])
                nc.sync.dma_start(
                    out[tt * 128 : (tt + 1) * 128, :], os_[:]
                )

        qkv_proj()
        for p in range(NP):
            attention(p)
        out_proj()


def make_in_maps(x, w_attn, w_proj):
    """Pack full fp32 inputs into per-core bf16 input maps."""
    bf = ml_dtypes.bfloat16
    x = np.asarray(x, dtype=np.float32)
    w_attn = np.asarray(w_attn, dtype=np.float32)
    w_proj = np.asarray(w_proj, dtype=np.float32)

    xbs = []
    for b in range(B):
        xT = x[b].T  # [D, T]
        xbs.append(
            np.ascontiguousarray(
                xT.reshape(KC, 128, NCH, NQ).transpose(1, 2, 0, 3)
            ).reshape(128, NCH * KC * NQ).astype(bf)
        )

    in_maps = []
    for c in range(NCORES):
        b, g = c // 2, c % 2
        c0 = g * CW
        wsl = np.concatenate(
            [w_attn[:, c0 : c0 + CW],
             w_attn[:, D + c0 : D + c0 + CW],
             w_attn[:, 2 * D + c0 : 2 * D + c0 + CW]],
            axis=1,
        )  # [D, 3*CW]
        wq = np.ascontiguousarray(
            wsl.reshape(KC, 128, NM, 128).transpose(1, 2, 0, 3)
        ).reshape(128, NM * KC * 128).astype(bf)
        wpc = np.ascontiguousarray(
            w_proj[c0 : c0 + CW, :].reshape(NP, 128, D).transpose(1, 0, 2)
        ).reshape(128, NP * D).astype(bf)
        in_maps.append({"xb": xbs[b], "wq": wq, "wp": wpc})
    return in_maps


_NC_CACHE = None


def kernel(x: np.ndarray, w_attn: np.ndarray, w_proj: np.ndarray) -> np.ndarray:
    global _NC_CACHE
    if _NC_CACHE is None:
        _NC_CACHE = build_kernel()
    nc = _NC_CACHE

    in_maps = make_in_maps(x, w_attn, w_proj)
    res = run_bass_kernel_spmd(nc, in_maps, core_ids=list(range(NCORES)))
    y = np.empty((B, T, D), dtype=np.float32)
    for b in range(B):
        y[b] = res.results[2 * b]["out"] + res.results[2 * b + 1]["out"]
    return y


if __name__ == "__main__":
    inputs = {
        "x": np.random.randn(B, T, D).astype(np.float32),
        "w_attn": (np.random.randn(D, 3 * D) / np.sqrt(D)).astype(np.float32),
        "w_proj": (np.random.randn(D, D) / np.sqrt(D)).astype(np.float32),
    }
    y = kernel(**inputs)
    print(y.shape, y.dtype)
